# revision 7
# baseline (speedup 1.0000x reference)
"""Per-channel Linear(seq->pred) over channels, 8-core channel-parallel Trainium2 kernel.

Math: y[b,p,c] = sum_s x[b,s,c] * W[c,p,s] + bias[c,p]

Strategy:
  - Shard channels C=321 across 8 cores (pad to 328 = 8*41).
  - Host-side re-layout (contraction padded to 726 = 6*121 rows):
      wt[c,s,p] = W[c,p,s] for s<720, wt[c,720,p] = bias[c,p], rows 721+ zero
      xt[c,s,b] = x[b,s,c] for s<720, xt[c,720,b] = 1.0,        rows 721+ zero
    so bias is folded into the contraction and the K dim splits into 6
    uniform chunks of 121 (one 3-dim DMA AP covers a whole channel pair).
  - Per channel: Y_c[b,p] = sum_k xT_chunk[k].T @ wT_chunk[k], accumulated in
    PSUM over the 6 K-chunks. lhsT = xT chunk [121,64] (stationary),
    rhs = wT chunk [121,720] streamed as N = 512 + 208 (PSUM bank limit).
  - Two channels share one PSUM tile via PE column tiling: channel A in
    output partitions 0:64, channel B in 64:128, matmuls interleaved so the
    two 64-wide column groups stream concurrently.
  - Result copied PSUM->SBUF (DVE + ACT split) and DMA'd out as y[c,b,p].
"""

import numpy as np

import concourse.bacc as bacc
import concourse.mybir as mybir
import concourse.tile as tile
from concourse.bass_utils import run_bass_kernel_spmd

F32 = mybir.dt.float32

B = 64          # batch
S = 720         # seq_len (contraction)
P = 720         # pred_len
C = 321         # channels
N_CORES = 8
CL = 41         # channels per core; 8*41 = 328 >= 321
CPAD = N_CORES * CL
KCH = 121       # K-chunk rows
NKCH = 6        # chunks per channel
SPAD = KCH * NKCH  # 726 padded contraction rows (720 data + bias + 5 zero)
NSPLIT = 512    # first matmul N (PSUM bank holds 512 f32)

_CACHE: dict = {}


def _build_module():
    nc = bacc.Bacc("TRN2", target_bir_lowering=False, debug=False,
                   num_devices=N_CORES)
    wt = nc.dram_tensor("wt", [CL, SPAD, P], F32, kind="ExternalInput").ap()
    xt = nc.dram_tensor("xt", [CL, SPAD, B], F32, kind="ExternalInput").ap()
    y = nc.dram_tensor("y", [CL, B, P], F32, kind="ExternalOutput").ap()

    with tile.TileContext(nc) as tc:
        with (
            tc.tile_pool(name="wp", bufs=3) as wp,
            tc.tile_pool(name="xp", bufs=3) as xp,
            tc.tile_pool(name="pp", bufs=3, space="PSUM") as pp,
            tc.tile_pool(name="op", bufs=3) as op,
        ):
            # process channels in pairs: two channels share one PSUM tile
            # (output partitions 0:64 and 64:128 -> PE column tiling).
            for c0 in range(0, CL, 2):
                pair = min(2, CL - c0)
                nch = pair * NKCH
                wbig = wp.tile([KCH, nch, P], F32, name=f"wbig{c0}", tag="wbig")
                xbig = xp.tile([KCH, nch, B], F32, name=f"xbig{c0}", tag="xbig")
                # (c, k) merge into one AP dim: c-step = SPAD*P = NKCH*(KCH*P)
                nc.sync.dma_start(
                    wbig[:],
                    wt[c0:c0 + pair].rearrange("c (k s) p -> s (c k) p", s=KCH))
                nc.sync.dma_start(
                    xbig[:],
                    xt[c0:c0 + pair].rearrange("c (k s) b -> s (c k) b", s=KCH))
                ps = pp.tile([pair * B, P], F32, name=f"ps{c0}", tag="ps")
                for k in range(NKCH):
                    st, sp = (k == 0), (k == NKCH - 1)
                    for half in range(pair):
                        ck = half * NKCH + k
                        lhsT = xbig[:, ck, :]
                        prow = half * B
                        nc.tensor.matmul(ps[prow:prow + B, 0:NSPLIT],
                                         lhsT, wbig[:, ck, 0:NSPLIT],
                                         start=st, stop=sp)
                        nc.tensor.matmul(ps[prow:prow + B, NSPLIT:P],
                                         lhsT, wbig[:, ck, NSPLIT:P],
                                         start=st, stop=sp)
                out = op.tile([pair * B, P], F32, name=f"out{c0}", tag="out")
                nc.vector.tensor_copy(out[:, 0:NSPLIT], ps[:, 0:NSPLIT])
                nc.scalar.copy(out[:, NSPLIT:P], ps[:, NSPLIT:P])
                nc.sync.dma_start(
                    y[c0:c0 + pair].rearrange("c b p -> (c b) p"), out[:])

    nc.compile()
    return nc


def _get_module():
    if "nc" not in _CACHE:
        _CACHE["nc"] = _build_module()
    return _CACHE["nc"]


def _prep_inputs(x, W, b):
    wt = np.zeros((CPAD, SPAD, P), dtype=np.float32)
    wt[:C, :S, :] = W.transpose(0, 2, 1)
    wt[:C, S, :] = b
    xt = np.zeros((CPAD, SPAD, B), dtype=np.float32)
    xt[:C, :S, :] = x.transpose(2, 1, 0)
    xt[:C, S, :] = 1.0
    in_maps = []
    for i in range(N_CORES):
        sl = slice(i * CL, (i + 1) * CL)
        in_maps.append({
            "wt": np.ascontiguousarray(wt[sl]),
            "xt": np.ascontiguousarray(xt[sl]),
        })
    return in_maps


def _gather(results):
    ys = np.concatenate([results[i]["y"] for i in range(N_CORES)], axis=0)
    return np.ascontiguousarray(ys[:C].transpose(1, 2, 0))


def run(x, W, b, **run_kwargs):
    """Full pipeline, returns (output, BassKernelResults)."""
    nc = _get_module()
    in_maps = _prep_inputs(np.asarray(x), np.asarray(W), np.asarray(b))
    res = run_bass_kernel_spmd(nc, in_maps, list(range(N_CORES)), **run_kwargs)
    return _gather(res.results), res


def kernel(x, W, b):
    out, _ = run(x, W, b)
    return out


# revision 8
# speedup vs baseline: 2.1565x; 2.1565x over previous
"""Per-channel Linear(seq->pred) over channels, 8-core channel-parallel Trainium2 kernel.

Math: y[b,p,c] = sum_s x[b,s,c] * W[c,p,s] + bias[c,p]

Strategy:
  - Shard channels C=321 across 8 cores (pad to 328 = 8*41).
  - Host-side re-layout (contraction padded to 726 = 6*121 rows):
      wt[c,s,p] = W[c,p,s] for s<720, wt[c,720,p] = bias[c,p], rows 721+ zero
      xt[c,s,b] = x[b,s,c] for s<720, xt[c,720,b] = 1.0,        rows 721+ zero
    so bias is folded into the contraction and the K dim splits into 6
    uniform chunks of 121 (one 3-dim DMA AP covers a whole channel pair).
  - Per channel: Y_c[b,p] = sum_k xT_chunk[k].T @ wT_chunk[k], accumulated in
    PSUM over the 6 K-chunks. lhsT = xT chunk [121,64] (stationary),
    rhs = wT chunk [121,720] streamed as N = 512 + 208 (PSUM bank limit).
  - Two channels share one PSUM tile via PE column tiling: channel A in
    output partitions 0:64, channel B in 64:128, matmuls interleaved so the
    two 64-wide column groups stream concurrently.
  - Result copied PSUM->SBUF (DVE + ACT split) and DMA'd out as y[c,b,p].
"""

import numpy as np

import concourse.bacc as bacc
import concourse.mybir as mybir
import concourse.tile as tile
from concourse.bass_utils import run_bass_kernel_spmd

F32 = mybir.dt.float32

B = 64          # batch
S = 720         # seq_len (contraction)
P = 720         # pred_len
C = 321         # channels
N_CORES = 8
CL = 41         # channels per core; 8*41 = 328 >= 321
CPAD = N_CORES * CL
KCH = 128       # K-chunk rows
NKCH = 6        # chunks per channel
SPAD = KCH * NKCH  # 726 padded contraction rows (720 data + bias + 5 zero)
NSPLIT = 512    # first matmul N (PSUM bank holds 512 f32)

_CACHE: dict = {}


def _build_module():
    nc = bacc.Bacc("TRN2", target_bir_lowering=False, debug=False,
                   num_devices=N_CORES)
    wt = nc.dram_tensor("wt", [CL, SPAD, P], F32, kind="ExternalInput").ap()
    xt = nc.dram_tensor("xt", [CL, SPAD, B], F32, kind="ExternalInput").ap()
    y = nc.dram_tensor("y", [CL, B, P], F32, kind="ExternalOutput").ap()

    with tile.TileContext(nc) as tc:
        with (
            tc.tile_pool(name="wp", bufs=3) as wp,
            tc.tile_pool(name="xp", bufs=3) as xp,
            tc.tile_pool(name="pp", bufs=3, space="PSUM") as pp,
            tc.tile_pool(name="op", bufs=3) as op,
        ):
            # process channels in pairs: two channels share one PSUM tile
            # (output partitions 0:64 and 64:128 -> PE column tiling).
            for c0 in range(0, CL, 2):
                pair = min(2, CL - c0)
                nch = pair * NKCH
                wbig = wp.tile([KCH, nch, P], F32, name=f"wbig{c0}", tag="wbig")
                xbig = xp.tile([KCH, nch, B], F32, name=f"xbig{c0}", tag="xbig")
                # (c, k) merge into one AP dim: c-step = SPAD*P = NKCH*(KCH*P)
                nc.sync.dma_start(
                    wbig[:],
                    wt[c0:c0 + pair].rearrange("c (k s) p -> s (c k) p", s=KCH))
                nc.sync.dma_start(
                    xbig[:],
                    xt[c0:c0 + pair].rearrange("c (k s) b -> s (c k) b", s=KCH))
                ps = pp.tile([pair * B, P], F32, name=f"ps{c0}", tag="ps")
                for k in range(NKCH):
                    st, sp = (k == 0), (k == NKCH - 1)
                    for half in range(pair):
                        ck = half * NKCH + k
                        lhsT = xbig[:, ck, :]
                        prow = half * B
                        nc.tensor.matmul(ps[prow:prow + B, 0:NSPLIT],
                                         lhsT, wbig[:, ck, 0:NSPLIT],
                                         start=st, stop=sp)
                        nc.tensor.matmul(ps[prow:prow + B, NSPLIT:P],
                                         lhsT, wbig[:, ck, NSPLIT:P],
                                         start=st, stop=sp)
                out = op.tile([pair * B, P], F32, name=f"out{c0}", tag="out")
                nc.vector.tensor_copy(out[:, 0:NSPLIT], ps[:, 0:NSPLIT])
                nc.scalar.copy(out[:, NSPLIT:P], ps[:, NSPLIT:P])
                nc.sync.dma_start(
                    y[c0:c0 + pair].rearrange("c b p -> (c b) p"), out[:])

    nc.compile()
    return nc


def _get_module():
    if "nc" not in _CACHE:
        _CACHE["nc"] = _build_module()
    return _CACHE["nc"]


def _prep_inputs(x, W, b):
    wt = np.zeros((CPAD, SPAD, P), dtype=np.float32)
    wt[:C, :S, :] = W.transpose(0, 2, 1)
    wt[:C, S, :] = b
    xt = np.zeros((CPAD, SPAD, B), dtype=np.float32)
    xt[:C, :S, :] = x.transpose(2, 1, 0)
    xt[:C, S, :] = 1.0
    in_maps = []
    for i in range(N_CORES):
        sl = slice(i * CL, (i + 1) * CL)
        in_maps.append({
            "wt": np.ascontiguousarray(wt[sl]),
            "xt": np.ascontiguousarray(xt[sl]),
        })
    return in_maps


def _gather(results):
    ys = np.concatenate([results[i]["y"] for i in range(N_CORES)], axis=0)
    return np.ascontiguousarray(ys[:C].transpose(1, 2, 0))


def run(x, W, b, **run_kwargs):
    """Full pipeline, returns (output, BassKernelResults)."""
    nc = _get_module()
    in_maps = _prep_inputs(np.asarray(x), np.asarray(W), np.asarray(b))
    res = run_bass_kernel_spmd(nc, in_maps, list(range(N_CORES)), **run_kwargs)
    return _gather(res.results), res


def kernel(x, W, b):
    out, _ = run(x, W, b)
    return out
